# revision 4
# baseline (speedup 1.0000x reference)
"""Multi-head attention block (B=4, S=2048, D=1024, H=16, DH=64) on 8 trn2 cores.

Sharding: tensor-parallel over heads (2 groups of 8) x data-parallel over batch (4).
Core c handles batch c//2, heads (c%2)*8 .. +8. Each core computes a partial
output projection (its 8 heads' contribution to cat @ W0) in fp16; the host sums
the two partials per batch and adds b0.

v3 layout/schedule (vs the v1 baseline):
  - x and Wq/Wk/Wv are bf16 (half the HBM traffic + host upload), W0 fp16,
    out fp16.
  - pair-major attention: after pair 0's q/k projection, all four q-blocks of
    pair 0 run (~74us of ACT exp work) while the PE projects pairs 1-3 in its
    slack, so the scalar engine starts ~16us into the kernel and never starves
    at pair transitions.
  - att(0,0) is split scores/exp -> vproj -> PV so the first exp lands before
    the v projection (trace order defines Tile dependencies).
  - PSUM budget: psA(proj) 2 banks + psS(scores) 4 + psPV 2 = 8; psA closes
    before psD(outproj) opens so outproj(qb) overlaps pair-3 attention.
  - attention per (qb, pair): scoresT[key, q] = kT.T @ qT per 128-key chunk,
    two heads row-paired on the PE array (K=64 each at base partition 0/64);
    exp on ACT (scale=1/8) -> fp16; PV via [v_h | 1] (M=65) accumulated over
    key chunks; PSUM evacuated immediately, then denominator row -> DMA shift
    -> reciprocal -> gpsimd broadcast -> multiply.
"""

import os
import sys

for _p in ("/opt/trn_rl_repo",):
    if _p not in sys.path and os.path.isdir(_p):
        sys.path.insert(0, _p)

import numpy as np

import concourse.bass as bass
import concourse.bacc as bacc_mod
import concourse.mybir as mybir
import concourse.tile as tile
import bass_rust
from concourse.vector_clock import ScopedClock

B, S, D, H, DH = 4, 2048, 1024, 16, 64
NCORES = 8
HL = 8            # heads per core
NP = HL // 2      # head pairs per core
E = HL * DH       # 512 local cat width
QB = 512          # q block (columns per attention block)
NQB = S // QB     # 4
KC = 128          # key chunk
NKC = S // KC     # 16
NDC = D // 128    # 8 contraction chunks for projections
F32 = mybir.dt.float32
F16 = mybir.dt.float16
BF16 = mybir.dt.bfloat16
EXPSCALE = 1.0 / np.sqrt(DH)

_MAXW = 1


def _patched_drain_and_barrier(self, tick_clock, wait_clock):
    """Walrus codegen only supports one sync-wait per CTRL instruction; Tile's
    stock exit drain piles every outstanding processor's sem wait onto a single
    drain. Split them across nops (same engine => program order preserved)."""
    probe = self.nc.sync.nop()
    wait_clock.add_sem_waits(probe.ins, ScopedClock({None: tick_clock.global_clock}))
    si = probe.ins.sync_info
    waits = list(si.on_wait) if si is not None and si.on_wait else []
    if len(waits) > _MAXW:
        probe.ins.sync_info = bass_rust.SyncInfo(on_wait=waits[:_MAXW], on_update=[])
        for i in range(_MAXW, len(waits), _MAXW):
            extra = self.nc.sync.nop()
            extra.ins.sync_info = bass_rust.SyncInfo(
                on_wait=waits[i : i + _MAXW], on_update=[]
            )
    self.nc.sync.drain()
    self.nc.all_engine_barrier()
    popped = self.nc._tile_sem_poison_stack.pop()
    assert popped is self._sem_poison
    self.nc.clear_and_free_semaphores(list(self.sems.allocated().values()))
    self.nc.all_engine_barrier()


tile.TileContext._drain_and_barrier = _patched_drain_and_barrier


def build_nc(reps=1):
    nc = bacc_mod.Bacc()
    xT = nc.dram_tensor("xT", [D, S], BF16, kind="ExternalInput")
    wq = nc.dram_tensor("wq", [D, E], BF16, kind="ExternalInput")
    wk = nc.dram_tensor("wk", [D, E], BF16, kind="ExternalInput")
    wv = nc.dram_tensor("wv", [D, E], BF16, kind="ExternalInput")
    bqk = nc.dram_tensor("bqk", [128, 2 * NP], F32, kind="ExternalInput")
    bvr = nc.dram_tensor("bvr", [1, E], F32, kind="ExternalInput")
    w0 = nc.dram_tensor("w0", [E, D], F16, kind="ExternalInput")
    out = nc.dram_tensor("out", [S, D], F16, kind="ExternalOutput")

    with tile.TileContext(nc) as tc:
        with tc.tile_pool(name="pwts", bufs=1) as pwts:
            # ---- persistent weights/biases: loaded once, read every rep ----
            bqkt = pwts.tile([128, 2 * NP], F32, tag="bqkt", name="bqkt")
            nc.sync.dma_start(bqkt[:], bqk[:])
            bvrow = pwts.tile([1, E], F32, tag="bvrow", name="bvrow")
            nc.sync.dma_start(bvrow[:], bvr[:])
            bvb = pwts.tile([128, E], F32, tag="bvb", name="bvb")
            nc.gpsimd.partition_broadcast(bvb[:], bvrow[:])

            w0t = []
            for p in range(NP):
                t = pwts.tile([128, D], F16, tag=f"w0_{p}", name=f"w0_{p}")
                nc.sync.dma_start(t[:], w0[p * 128 : (p + 1) * 128, :])
                w0t.append(t)

            def load_w(dram, prefix):
                ts = []
                for k in range(NDC):
                    t = pwts.tile([128, E], BF16, tag=f"{prefix}{k}",
                                  name=f"{prefix}{k}")
                    nc.sync.dma_start(t[:], dram[k * 128 : (k + 1) * 128, :])
                    ts.append(t)
                return ts

            wk_t = load_w(wk, "wk")
            wq_t = load_w(wq, "wq")
            wv_t = load_w(wv, "wv")

            for _rep in range(reps):
              with (
                tc.tile_pool(name="pqkt", bufs=1) as pqkt,
                tc.tile_pool(name="pcat", bufs=1) as pcat,
                tc.tile_pool(name="pv", bufs=1) as pvpool,
                tc.tile_pool(name="pexp", bufs=7) as pexp,
                tc.tile_pool(name="psm", bufs=2) as psm,
                tc.tile_pool(name="psS", bufs=1, space="PSUM") as psS_pool,
                tc.tile_pool(name="psPV", bufs=1, space="PSUM") as psPV_pool,
              ):
                catq = [
                    [pcat.tile([128, QB], F16, tag=f"cat{p}_{qb}", name=f"cat{p}_{qb}")
                     for qb in range(NQB)]
                    for p in range(NP)
                ]
                vaug = [
                    pvpool.tile([128, HL * 65], F16, tag=f"v{sc}", name=f"v{sc}")
                    for sc in range(NKC)
                ]
                qt = [[None] * NQB for _ in range(NP)]
                kt = [[None] * NQB for _ in range(NP)]

                def scores_exp(qb, p, kcg):
                    psS = [
                        psS_pool.tile([128, 1024], F32, tag=f"s{sub}",
                                      name=f"s{sub}")
                        for sub in range(2)
                    ]
                    for j in range(2):
                        kc = kcg * 2 + j
                        ktile = kt[p][kc // 4]
                        ksl = slice((kc % 4) * 128, (kc % 4) * 128 + 128)
                        for sub in range(2):
                            rows = slice(sub * 64, sub * 64 + 64)
                            nc.tensor.matmul(
                                psS[sub][:, j * QB : (j + 1) * QB],
                                ktile[rows, ksl],
                                qt[p][qb][rows, :],
                                start=True,
                                stop=True,
                            )
                    et = [
                        pexp.tile([128, 1024], F16, tag=f"e{sub}",
                                  name=f"e{sub}")
                        for sub in range(2)
                    ]
                    for sub in range(2):
                        nc.scalar.activation(
                            et[sub][:],
                            psS[sub][:],
                            mybir.ActivationFunctionType.Exp,
                            scale=EXPSCALE,
                        )
                    return et

                def pv_acc(p, pv, kcg, et):
                    for j in range(2):
                        kc = kcg * 2 + j
                        for sub in range(2):
                            h = p * 2 + sub
                            nc.tensor.matmul(
                                pv[sub][:],
                                vaug[kc][:, h * 65 : (h + 1) * 65],
                                et[sub][:, j * QB : (j + 1) * QB],
                                start=(kc == 0),
                                stop=(kc == NKC - 1),
                            )

                def norm(qb, p, pv):
                    # normalize: row 64 of pv = softmax denominator.
                    # evacuate PSUM first (frees the pv bank for the next
                    # unit's accumulation), then normalize from SBUF.
                    for sub in range(2):
                        pvs = psm.tile([65, QB], F32, tag="pvs", name="pvs")
                        nc.vector.tensor_copy(pvs[:], pv[sub][:])
                        srow = psm.tile([1, QB], F32, tag="srow", name="srow")
                        nc.sync.dma_start(srow[:], pvs[64:65, :])
                        rrow = psm.tile([1, QB], F32, tag="rrow", name="rrow")
                        nc.vector.reciprocal_approx_fast(rrow[:], srow[:])
                        rb = psm.tile([64, QB], F32, tag="rb", name="rb")
                        nc.gpsimd.partition_broadcast(rb[:], rrow[:])
                        if sub == 0:
                            nc.vector.tensor_mul(
                                catq[p][qb][0:64, :], pvs[0:64, :], rb[:]
                            )
                        else:
                            tb = psm.tile([64, QB], F16, tag="tb", name="tb")
                            nc.vector.tensor_mul(tb[:], pvs[0:64, :], rb[:])
                            nc.sync.dma_start(catq[p][qb][64:128, :], tb[:])

                # software pipeline: unit(n)'s scores/exp interleave with
                # unit(n-1)'s PV accumulation at key-chunk granularity, and
                # unit(n-1)'s normalize overlaps unit(n)'s scores.
                pending = []

                def unit(qb_p):
                    cur = None
                    if qb_p is not None:
                        qb, p = qb_p
                        pv = [
                            psPV_pool.tile([65, QB], F32, tag=f"pv{sub}",
                                           name=f"pv{sub}")
                            for sub in range(2)
                        ]
                        cur = (qb, p, [], pv)
                    prev = pending[0] if pending else None
                    for kcg in range(NKC // 2):
                        if cur is not None:
                            cur[2].append(scores_exp(cur[0], cur[1], kcg))
                        if prev is not None:
                            pv_acc(prev[1], prev[3], kcg, prev[2][kcg])
                    if prev is not None:
                        norm(prev[0], prev[1], prev[3])
                    pending.clear()
                    if cur is not None:
                        pending.append(cur)
                    return prev

                with (
                    tc.tile_pool(name="pxt", bufs=32) as pxt,
                    tc.tile_pool(name="psA", bufs=2, space="PSUM") as psA,
                ):
                    xts = [[None] * NQB for _ in range(NDC)]
                    for sb in range(NQB):
                        for k in range(NDC):
                            t = pxt.tile([128, QB], BF16, tag="xt", name="xt")
                            nc.sync.dma_start(
                                t[:],
                                xT[k * 128 : (k + 1) * 128,
                                   sb * QB : (sb + 1) * QB],
                            )
                            xts[k][sb] = t

                    def proj_qk(p):
                        for wt, bias_col, dest in (
                            (wk_t, NP, kt), (wq_t, 0, qt)
                        ):
                            for sb in range(NQB):
                                ps = psA.tile([128, QB], F32, tag="ps", name="ps")
                                for k in range(NDC):
                                    nc.tensor.matmul(
                                        ps[:],
                                        wt[k][:, p * 128 : (p + 1) * 128],
                                        xts[k][sb][:],
                                        start=(k == 0),
                                        stop=(k == NDC - 1),
                                    )
                                t = pqkt.tile(
                                    [128, QB], F16,
                                    tag=f"qk{dest is kt}{p}{sb}", name="qkt"
                                )
                                nc.vector.tensor_scalar_add(
                                    t[:], ps[:],
                                    bqkt[:, bias_col + p : bias_col + p + 1]
                                )
                                dest[p][sb] = t

                    def vproj():
                        for sc in range(NKC):
                            ps = psA.tile([128, E], F32, tag="ps", name="ps")
                            for k in range(NDC):
                                nc.tensor.matmul(
                                    ps[:],
                                    xts[k][sc // 4][
                                        :, (sc % 4) * 128 : (sc % 4 + 1) * 128
                                    ],
                                    wv_t[k][:],
                                    start=(k == 0),
                                    stop=(k == NDC - 1),
                                )
                            va = vaug[sc]
                            nc.gpsimd.memset(
                                va[:].rearrange("p (h c) -> p h c", c=65)[
                                    :, :, 64:65
                                ],
                                1.0,
                            )
                            nc.vector.tensor_add(
                                va[:].rearrange("p (h c) -> p h c", c=65)[
                                    :, :, 0:64
                                ],
                                ps[:].rearrange("p (h c) -> p h c", c=64),
                                bvb[:].rearrange("p (h c) -> p h c", c=64),
                            )

                    proj_qk(0)
                    unit((0, 0))
                    vproj()
                    unit((1, 0))
                    proj_qk(1)
                    unit((2, 0))
                    unit((3, 0))
                    unit((0, 1))
                    proj_qk(2)
                    for qb in range(1, NQB):
                        unit((qb, 1))
                    unit((0, 2))
                    proj_qk(3)
                    for qb in range(1, NQB):
                        unit((qb, 2))

                with (
                    tc.tile_pool(name="pout", bufs=4) as pout,
                    tc.tile_pool(name="psD", bufs=2, space="PSUM") as psD,
                ):
                    def outproj(qb):
                        for sc4 in range(4):
                            for db in range(D // QB):
                                ps = psD.tile([128, QB], F32, tag="po",
                                              name="po")
                                for p in range(NP):
                                    nc.tensor.matmul(
                                        ps[:],
                                        catq[p][qb][:, sc4 * 128 : (sc4 + 1) * 128],
                                        w0t[p][:, db * QB : (db + 1) * QB],
                                        start=(p == 0),
                                        stop=(p == NP - 1),
                                    )
                                ot = pout.tile([128, QB], F16, tag="ot",
                                               name="ot")
                                nc.vector.tensor_copy(ot[:], ps[:])
                                sc = qb * 4 + sc4
                                nc.sync.dma_start(
                                    out[sc * 128 : (sc + 1) * 128,
                                        db * QB : (db + 1) * QB],
                                    ot[:],
                                )

                    for qb in range(NQB):
                        prev = unit((qb, 3))
                        if prev is not None and prev[1] == 3:
                            outproj(prev[0])
                    prev = unit(None)
                    outproj(prev[0])
    nc.finalize()
    return nc


_NC_CACHE = None


def _get_nc():
    global _NC_CACHE
    if _NC_CACHE is None:
        _NC_CACHE = build_nc()
    return _NC_CACHE


def make_in_maps(x, Wq, bq, Wk, bk, Wv, bv, W0, b0):
    import ml_dtypes

    bf16 = ml_dtypes.bfloat16
    x = np.asarray(x, dtype=np.float32)
    xTb = [np.ascontiguousarray(x[b].T).astype(bf16) for b in range(B)]
    # per TP half (shared by the 4 cores handling different batches)
    half = {}
    for t in range(2):
        h0 = t * HL
        sl = slice(h0, h0 + HL)
        wq_c = np.ascontiguousarray(
            np.asarray(Wq[sl], np.float32).transpose(1, 0, 2).reshape(D, E)
        ).astype(bf16)
        wk_c = np.ascontiguousarray(
            np.asarray(Wk[sl], np.float32).transpose(1, 0, 2).reshape(D, E)
        ).astype(bf16)
        wv_c = np.ascontiguousarray(
            np.asarray(Wv[sl], np.float32).transpose(1, 0, 2).reshape(D, E)
        ).astype(bf16)
        bq_c = np.asarray(bq[sl], np.float32).reshape(E)
        bk_c = np.asarray(bk[sl], np.float32).reshape(E)
        bqk_c = np.empty((128, 2 * NP), np.float32)
        for g in range(NP):
            bqk_c[:, g] = bq_c[g * 128 : (g + 1) * 128]
            bqk_c[:, NP + g] = bk_c[g * 128 : (g + 1) * 128]
        bv_c = np.asarray(bv[sl], np.float32).reshape(1, E)
        w0_c = np.ascontiguousarray(
            np.asarray(W0[h0 * DH : (h0 + HL) * DH], np.float32)
        ).astype(np.float16)
        half[t] = dict(wq=wq_c, wk=wk_c, wv=wv_c, bqk=bqk_c, bvr=bv_c, w0=w0_c)
    in_maps = []
    for c in range(NCORES):
        b = c // 2
        t = c % 2
        in_maps.append({"xT": xTb[b], **half[t]})
    return in_maps


def combine(results, b0):
    out = np.empty((B, S, D), np.float32)
    for b in range(B):
        out[b] = np.asarray(results[2 * b]["out"], np.float32)
        out[b] += np.asarray(results[2 * b + 1]["out"], np.float32)
    out += np.asarray(b0, np.float32)[None, None, :]
    return out


def kernel(x, Wq, bq, Wk, bk, Wv, bv, W0, b0):
    from concourse.bass_utils import run_bass_kernel_spmd

    nc = _get_nc()
    in_maps = make_in_maps(x, Wq, bq, Wk, bk, Wv, bv, W0, b0)
    res = run_bass_kernel_spmd(nc, in_maps, core_ids=list(range(NCORES)))
    return combine(res.results, b0)
